# revision 24
# baseline (speedup 1.0000x reference)
"""Per-channel EMA (first-order linear recurrence along time) on 8 TRN2 cores.

  y[b, c, 0] = x[b, c, 0]
  y[b, c, t] = (1 - alpha[c]) * y[b, c, t-1] + alpha[c] * x[b, c, t]

Strategy (v5: radix-2 scan + PE offload, bf16 IO, packed layout, 2+2 queues)
  - Data-parallel over batch: B=32 -> 4 batches per core, alpha replicated.
  - Per core: 16 tiles of [128 channels (partitions), 2048 time (free)].
  - The DVE tensor_tensor_scan runs at ~2.1 cycles/element regardless of
    dtype, so a full-tile scan costs ~4.4us and 16 of them (~86us) dominated
    the v1 kernel. Here the recurrence is decimated by 2:
        even outputs:  z_m = y_{2m} = d^2 * z_{m-1} + u_m,
                       u_m = a*d*x_{2m-1} + a*x_{2m}   (u_0 = x_0)
        odd outputs:   y_{2m+1} = d * z_m + a * x_{2m+1}
    The DVE scans only the 1024 even columns (~2.3us/tile); u and the odd
    reconstruction are diagonal matmuls on the otherwise-idle Tensor engine
    (per-channel scale = diag weight matrix, PSUM f32 accumulation):
        u    = diag(a*d) @ x_odd<<1 + diag(a) @ x_even  (+ diag(d) @ x_0 on
               column 0, making u_0 = (a+d)*x_0 = x_0 exactly)
        y_od = diag(d) @ z + diag(a) @ x_odd
    The ACT engine copies the odd PSUM back to SBUF. Diag weights and d^2
    (fp32) are precomputed on host from alpha - alpha is a kernel input.
  - Everything HBM-facing is bf16: halves the 32 MiB/core round trip AND
    runs the PE at its fast rate (fp16 matmuls measured ~2x slower). The
    scan state stays fp32 internally with d^2 in fp32, so the recurrence
    does not accumulate quantization error (|d|<1 contraction; measured rel
    err ~2e-3, gate is 2e-2).
  - Host packs each core's x into [128, 16*2048] bf16: tile-major, each tile
    block holding its 1024 even time steps then its 1024 odd ones. Every
    device access is contiguous and every load is one contiguous-per-
    partition DMA (~144 descriptors).
  - Queue discipline (each HWDGE trigger costs ~0.65us of its engine queue,
    and a queue serializes its own transfers, so saturating HBM needs two
    load queues + two store queues):
      * the PE queue wakes earliest (~5us): tile-0's first half + x1 ride
        nc.tensor.dma_start as its first instructions, and odd tiles'
        loads interleave into the PE queue with 2-tile lookahead;
      * even tiles' loads ride the SP (sync) queue;
      * weights/d2 + tile-0's second half ride the ACT ring ahead of the
        copies; the ACT-table warm-up op sits after them;
      * stores alternate SWDGE-on-GpSimd / ACT ring; the last three tiles
        all use the ring (SWDGE completion lags ~10us and would push out
        the kernel drain).
  - PE is emitted with a one-tile skew (BCA_{n+1} before DE_n) so it
    pipelines around the scan; tile 0 runs as two chained half-chunks, the
    last tile as four chained quarter-chunks whose [ev|od] blocks store
    contiguously (quarter-interleaved y block), pulling the drain forward.
"""

import numpy as np
import ml_dtypes

import concourse.bass as bass
import concourse.bacc as bacc
import concourse.mybir as mybir
from concourse.tile import TileContext
from concourse.bass_utils import run_bass_kernel_spmd

B, C, L = 32, 512, 2048
N_CORES = 8
B_SH = B // N_CORES  # 4 batches per core
P = 128              # SBUF partitions
N_CB = C // P        # 4 channel blocks
N_TILES = B_SH * N_CB
LH = L // 2          # 1024 scan columns per tile
LQ = LH // 4         # 256-column quarters for the last tile

_F32 = mybir.dt.float32
_BF16 = mybir.dt.bfloat16
_NP_BF16 = ml_dtypes.bfloat16

mult = mybir.AluOpType.mult
add = mybir.AluOpType.add

LAST = N_TILES - 1


def build_nc() -> bass.Bass:
    # Bacc (not raw Bass): its compile() runs generate_event_semaphores,
    # which splits multi-sem waits — TRN2 allows at most one wait command
    # per instruction, and Tile freely emits several.
    nc = bacc.Bacc()
    x = nc.dram_tensor("x", [P, N_TILES * L], _BF16, kind="ExternalInput")
    # w[p, (cb*3+j)*P + m]: diag weight blocks, j=0: diag(a), 1: diag(a*d),
    # 2: diag(d) for channel block cb (built on host, already in SBUF layout)
    w = nc.dram_tensor("w", [P, N_CB * 3 * P], _BF16, kind="ExternalInput")
    d2 = nc.dram_tensor("d2", [P, N_CB], _F32, kind="ExternalInput")
    y = nc.dram_tensor("y", [P, N_TILES * L], _BF16, kind="ExternalOutput")

    with TileContext(nc) as tc:
        with (
            tc.tile_pool(name="xp", bufs=5) as xp,
            tc.tile_pool(name="yp", bufs=6) as yp,
            tc.tile_pool(name="cp", bufs=1) as cp,
            tc.tile_pool(name="up", bufs=2, space="PSUM") as up,
            tc.tile_pool(name="wp", bufs=2, space="PSUM") as wp,
        ):
            xv = [None] * N_TILES  # per-tile [P, L] x views (ev | od)
            yv = [None] * N_TILES  # per-tile [P, L] y views
            tiles_u = [None] * N_TILES
            tiles_w = [None] * N_TILES

            def alloc(n):
                yv[n] = yp.tile([P, L], _BF16, tag="y", name=f"yt{n}", bufs=8)
                tiles_u[n] = up.tile([P, LH], _F32, tag="u", name="u")
                tiles_w[n] = wp.tile([P, LH], _F32, tag="w", name="wv")

            def load(n, engine, k=1):
                # every tile gets its own buffer: a load trigger that waits
                # for a pool release blocks its whole DMA queue, and the
                # resulting load-lag feeds back into scan-lag
                g = xp.tile(
                    [P, k * L], _BF16, tag=f"x{k}", name=f"xt{n}",
                    bufs=(10 if k == 1 else 3),
                )
                engine.dma_start(out=g, in_=x[:, n * L : (n + k) * L])
                for i in range(k):
                    xv[n + i] = g[:, i * L : (i + 1) * L]

            # ---- startup. Each queue serializes trigger+transfer AND a
            # ~2.2us/DMA completion pipeline, and no completion lands before
            # ~12us - so the critical first inputs are spread across all
            # three DMA paths, one deep each:
            #   sync:   d2 (gates scan_0), then x1, x2, ...
            #   swdge:  weights (gate BCA_0)
            #   ring:   tile-0's two interleaved halves
            d2t = cp.tile([P, N_CB], _F32, tag="d2t", name="d2t")
            nc.sync.dma_start(out=d2t, in_=d2[:, :])
            wt = cp.tile([P, N_CB * 3 * P], _BF16, tag="wt", name="wt")
            nc.gpsimd.dma_start(out=wt, in_=w[:, :])
            g0 = xp.tile([P, L], _BF16, tag="x1", name="xt0", bufs=10)
            xv[0] = g0
            nc.scalar.dma_start(out=g0[:, 0:LH], in_=x[:, 0:LH])
            nc.scalar.dma_start(out=g0[:, LH:L], in_=x[:, LH:L])
            # warm-up ACT op after the triggers: its d2-completion wait must
            # not stall the queued x0 loads; it still runs before copy_0
            warm = cp.tile([P, N_CB], _F32, tag="warm", name="warm")
            nc.scalar.mul(warm, d2t, 1.0)
            # SP queue: x1, x2, singles 3..9, then pairs to compress the tail
            load(1, nc.sync)
            load(2, nc.sync)
            for n in range(3, 10):
                load(n, nc.sync)
            load(10, nc.sync, k=2)
            load(12, nc.sync, k=2)
            load(14, nc.sync, k=2)

            def W(cb, j):
                o = (cb * 3 + j) * P
                return wt[:, o : o + P]

            HB = LH // 2  # 512-column halves

            # tile-15's y block and tile-0's x block are half-interleaved
            # [ev_h0 | od_h0 | ev_h1 | od_h1]; others are [ev | od]
            def zcols(n, lo, hi):
                if n == LAST:
                    k = lo // HB
                    assert (hi - 1) // HB == k
                    return yv[n][:, lo + k * HB : hi + k * HB]
                return yv[n][:, lo:hi]

            def ocols(n, lo, hi):
                if n == LAST:
                    k = lo // HB
                    assert (hi - 1) // HB == k
                    return yv[n][:, lo + (k + 1) * HB : hi + (k + 1) * HB]
                return yv[n][:, LH + lo : LH + hi]

            def xev(n, lo, hi):
                if n == 0:
                    k = lo // HB
                    assert (hi - 1) // HB == k
                    return xv[n][:, lo + k * HB : hi + k * HB]
                return xv[n][:, lo:hi]

            def xod(n, lo, hi):
                if n == 0:
                    k = lo // HB
                    assert (hi - 1) // HB == k
                    return xv[n][:, lo + (k + 1) * HB : hi + (k + 1) * HB]
                return xv[n][:, LH + lo : LH + hi]

            def emit_bca(n, lo, hi):
                """u[lo:hi] = diag(a)@x_ev[lo:hi] + diag(ad)@x_od[lo-1:hi-1]
                (+ diag(d)@x_0 on column 0), emitted as <=512-col matmuls."""
                u = tiles_u[n]
                cb = n // B_SH
                Wa, Wad, Wd = W(cb, 0), W(cb, 1), W(cb, 2)
                for s in range(lo, hi, 512):
                    e = min(s + 512, hi)
                    nc.tensor.matmul(
                        out=u[:, s:e], lhsT=Wa, rhs=xev(n, s, e),
                        start=True, stop=False,
                    )
                for s in range(lo, hi, 512):
                    e = min(s + 512, hi)
                    s2 = max(s, 1)  # u_0 has no x_{-1} term
                    # the shifted odd read od[s2-1:e-1] may straddle an
                    # interleave boundary on tile 0: emit per straddle piece.
                    # stop on the final writer of each PSUM region: regions
                    # containing column 0 are finished by the A-matmul below
                    pieces = [(s2, e)]
                    if n == 0 and (s2 - 1) // HB != (e - 2) // HB:
                        mid = ((s2 - 1) // HB + 1) * HB + 1
                        pieces = [(s2, mid), (mid, e)]
                    for i, (ps, pe) in enumerate(pieces):
                        nc.tensor.matmul(
                            out=u[:, ps:pe], lhsT=Wad,
                            rhs=xod(n, ps - 1, pe - 1),
                            start=False,
                            stop=(s > 0) and (i == len(pieces) - 1),
                        )
                if lo == 0:
                    nc.tensor.matmul(
                        out=u[:, 0:1], lhsT=Wd, rhs=xev(n, 0, 1),
                        start=False, stop=True,
                    )

            def emit_scan(n, lo, hi):
                cb = n // B_SH
                if lo == 0:
                    init = 0.0
                else:
                    init = zcols(n, lo - 1, lo)
                nc.vector.tensor_tensor_scan(
                    out=zcols(n, lo, hi),
                    data0=d2t[:, cb : cb + 1].broadcast_to([P, hi - lo]),
                    data1=tiles_u[n][:, lo:hi],
                    initial=init,
                    op0=mult,
                    op1=add,
                )

            def emit_de(n, lo, hi):
                """wv[lo:hi] = diag(d) @ z[lo:hi] + diag(a) @ x_od[lo:hi]"""
                wv = tiles_w[n]
                cb = n // B_SH
                Wa, Wd = W(cb, 0), W(cb, 2)
                for s in range(lo, hi, 512):
                    e = min(s + 512, hi)
                    nc.tensor.matmul(
                        out=wv[:, s:e], lhsT=Wd, rhs=zcols(n, s, e),
                        start=True, stop=False,
                    )
                for s in range(lo, hi, 512):
                    e = min(s + 512, hi)
                    nc.tensor.matmul(
                        out=wv[:, s:e], lhsT=Wa, rhs=xod(n, s, e),
                        start=False, stop=True,
                    )

            def emit_copy(n, lo, hi):
                nc.scalar.copy(ocols(n, lo, hi), tiles_w[n][:, lo:hi])

            def emit_store(m):
                # spread stores across queues; the LAST stores use sync+ring
                # (one each) so their ~2.2us completion handling overlaps
                if m == N_TILES - 3:
                    dma = nc.sync.dma_start  # sync is done loading by now
                elif m % 2 == 1 or m == N_TILES - 2:
                    dma = nc.scalar.dma_start
                else:
                    dma = nc.gpsimd.dma_start
                dma(out=y[:, m * L : (m + 1) * L], in_=yv[m])

            # tiles 0 and 1: chained half-chunks (the pipeline head is
            # gated by per-DMA completion latency; half-chunks let each
            # scan start one half-load earlier)
            alloc(0)
            for c in range(2):
                lo, hi = c * HB, (c + 1) * HB
                emit_bca(0, lo, hi)
                emit_scan(0, lo, hi)

            for n in range(1, LAST):
                alloc(n)
                if n == 1:
                    for c in range(2):
                        lo, hi = c * HB, (c + 1) * HB
                        emit_bca(n, lo, hi)
                        emit_scan(n, lo, hi)
                    emit_de(n - 1, 0, LH)
                    emit_copy(n - 1, 0, LH)
                    emit_store(n - 1)
                    continue
                emit_bca(n, 0, LH)
                emit_de(n - 1, 0, LH)
                emit_scan(n, 0, LH)
                emit_copy(n - 1, 0, LH)
                emit_store(n - 1)

            # last tile: two chained half-chunks; each half's [ev|od]
            # block stores contiguously on the ACT ring, and each half's
            # odd PSUM region is a whole bank (quarter-granularity DE/copy
            # serialized on PSUM bank accumulation groups)
            alloc(LAST)
            emit_bca(LAST, 0, LH)
            emit_de(LAST - 1, 0, LH)
            emit_scan(LAST, 0, HB)
            emit_scan(LAST, HB, LH)
            emit_copy(LAST - 1, 0, LH)
            emit_store(LAST - 1)
            for k in range(2):
                lo, hi = k * HB, (k + 1) * HB
                emit_de(LAST, lo, hi)
                emit_copy(LAST, lo, hi)
                dma = nc.sync.dma_start if k == 0 else nc.scalar.dma_start
                dma(
                    out=y[:, LAST * L + 2 * lo : LAST * L + 2 * hi],
                    in_=yv[LAST][:, 2 * lo : 2 * hi],
                )

    nc.compile()
    return nc


def _host_consts(alpha: np.ndarray):
    """Diag weight blocks (bf16, SBUF layout) + d^2 (fp32) from alpha."""
    a = alpha[0].astype(np.float64)  # [C]
    d = 1.0 - a
    # bf16 diag entries; d16 = 1 - a16 in bf16 arithmetic so the u_0 column
    # fixup (a16 + d16) lands as close to exactly 1 as bf16 allows
    a16 = a.astype(_NP_BF16)
    d16 = (_NP_BF16(1.0) - a16).astype(_NP_BF16)
    ad16 = (a16 * d16).astype(_NP_BF16)
    w = np.zeros((P, N_CB * 3 * P), dtype=_NP_BF16)
    idx = np.arange(P)
    for cb in range(N_CB):
        s = slice(cb * P, (cb + 1) * P)
        for j, v in enumerate((a16[s], ad16[s], d16[s])):
            w[idx, (cb * 3 + j) * P + idx] = v
    # d2 pre-arranged to the device SBUF layout [P, N_CB] (column j =
    # channel block j), so the load is one contiguous DMA
    d2 = np.ascontiguousarray(
        (d * d).astype(np.float32).reshape(N_CB, P).T
    )
    return w, d2


def _pack_core(xc: np.ndarray) -> np.ndarray:
    """[B_SH, C, L] -> [P, N_TILES*L] bf16: tile n = (cb*B_SH + b), block
    layout [evens | odds]."""
    x5 = xc.reshape(B_SH, N_CB, P, LH, 2)        # b, cb, p, m, parity
    x5 = x5.transpose(2, 1, 0, 4, 3)             # p, cb, b, parity, m
    out = np.ascontiguousarray(
        x5.astype(_NP_BF16)
    ).reshape(P, N_TILES * L)
    # tile 0's block becomes half-interleaved [ev_h0|od_h0|ev_h1|od_h1]
    t0 = out[:, 0:L].reshape(P, 2, 2, LH // 2)   # p, parity, half, m
    out[:, 0:L] = np.ascontiguousarray(
        t0.transpose(0, 2, 1, 3)
    ).reshape(P, L)
    return out


def _unpack_core(yc: np.ndarray) -> np.ndarray:
    """Inverse of _pack_core (+ the last tile's quarter-interleaved block)."""
    yc = yc.reshape(P, N_TILES, L).astype(np.float32)
    std = np.empty((P, N_TILES, 2, LH), dtype=np.float32)
    std[:, :, 0, :] = yc[:, :, 0:LH]
    std[:, :, 1, :] = yc[:, :, LH:L]
    lasth = yc[:, LAST].reshape(P, 2, 2, LH // 2)  # p, half, parity, m
    std[:, LAST, 0] = lasth[:, :, 0].reshape(P, LH)
    std[:, LAST, 1] = lasth[:, :, 1].reshape(P, LH)
    # std: p, (cb b), parity, m  ->  b, cb*P+p, 2m+parity
    y5 = std.reshape(P, N_CB, B_SH, 2, LH).transpose(2, 1, 0, 4, 3)
    return np.ascontiguousarray(y5).reshape(B_SH, C, L)


_cached_nc = None


def _get_nc() -> bass.Bass:
    global _cached_nc
    if _cached_nc is None:
        _cached_nc = build_nc()
    return _cached_nc


def run(x: np.ndarray, alpha: np.ndarray, nc=None, **spmd_kwargs):
    """Full host path: prep inputs, run on 8 cores, reassemble output.
    Returns (y, BassKernelResults)."""
    assert x.shape == (B, C, L) and alpha.shape == (1, C)
    x = np.asarray(x, dtype=np.float32)
    alpha = np.ascontiguousarray(alpha, dtype=np.float32)
    w, d2 = _host_consts(alpha)
    if nc is None:
        nc = _get_nc()
    in_maps = [
        {"x": _pack_core(x[c * B_SH : (c + 1) * B_SH]), "w": w, "d2": d2}
        for c in range(N_CORES)
    ]
    res = run_bass_kernel_spmd(nc, in_maps, list(range(N_CORES)), **spmd_kwargs)
    y = np.concatenate([_unpack_core(r["y"]) for r in res.results], axis=0)
    return y, res


def kernel(x: np.ndarray, alpha: np.ndarray) -> np.ndarray:
    return run(x, alpha)[0]


# revision 25
# speedup vs baseline: 1.0794x; 1.0794x over previous
"""Per-channel EMA (first-order linear recurrence along time) on 8 TRN2 cores.

  y[b, c, 0] = x[b, c, 0]
  y[b, c, t] = (1 - alpha[c]) * y[b, c, t-1] + alpha[c] * x[b, c, t]

Strategy (v5: radix-2 scan + PE offload, bf16 IO, packed layout, 2+2 queues)
  - Data-parallel over batch: B=32 -> 4 batches per core, alpha replicated.
  - Per core: 16 tiles of [128 channels (partitions), 2048 time (free)].
  - The DVE tensor_tensor_scan runs at ~2.1 cycles/element regardless of
    dtype, so a full-tile scan costs ~4.4us and 16 of them (~86us) dominated
    the v1 kernel. Here the recurrence is decimated by 2:
        even outputs:  z_m = y_{2m} = d^2 * z_{m-1} + u_m,
                       u_m = a*d*x_{2m-1} + a*x_{2m}   (u_0 = x_0)
        odd outputs:   y_{2m+1} = d * z_m + a * x_{2m+1}
    The DVE scans only the 1024 even columns (~2.3us/tile); u and the odd
    reconstruction are diagonal matmuls on the otherwise-idle Tensor engine
    (per-channel scale = diag weight matrix, PSUM f32 accumulation):
        u    = diag(a*d) @ x_odd<<1 + diag(a) @ x_even  (+ diag(d) @ x_0 on
               column 0, making u_0 = (a+d)*x_0 = x_0 exactly)
        y_od = diag(d) @ z + diag(a) @ x_odd
    The ACT engine copies the odd PSUM back to SBUF. Diag weights and d^2
    (fp32) are precomputed on host from alpha - alpha is a kernel input.
  - Everything HBM-facing is fp16: halves the 32 MiB/core round trip
    (measured: scan, PE and ACT rates are dtype-blind, so 16-bit IO is
    free). The scan state stays fp32 internally with d^2 in fp32, so the
    recurrence does not accumulate quantization error (|d|<1 contraction;
    measured rel err ~4.5e-4, gate is 2e-2).
  - Host packs each core's x into [128, 16*2048] bf16: tile-major, each tile
    block holding its 1024 even time steps then its 1024 odd ones. Every
    device access is contiguous and every load is one contiguous-per-
    partition DMA (~144 descriptors).
  - Queue discipline (each HWDGE trigger costs ~0.65us of its engine queue,
    and a queue serializes its own transfers, so saturating HBM needs two
    load queues + two store queues):
      * the PE queue wakes earliest (~5us): tile-0's first half + x1 ride
        nc.tensor.dma_start as its first instructions, and odd tiles'
        loads interleave into the PE queue with 2-tile lookahead;
      * even tiles' loads ride the SP (sync) queue;
      * weights/d2 + tile-0's second half ride the ACT ring ahead of the
        copies; the ACT-table warm-up op sits after them;
      * stores alternate SWDGE-on-GpSimd / ACT ring; the last three tiles
        all use the ring (SWDGE completion lags ~10us and would push out
        the kernel drain).
  - PE is emitted with a one-tile skew (BCA_{n+1} before DE_n) so it
    pipelines around the scan; tile 0 runs as two chained half-chunks, the
    last tile as four chained quarter-chunks whose [ev|od] blocks store
    contiguously (quarter-interleaved y block), pulling the drain forward.
"""

import numpy as np

import concourse.bass as bass
import concourse.bacc as bacc
import concourse.mybir as mybir
from concourse.tile import TileContext
from concourse.bass_utils import run_bass_kernel_spmd

B, C, L = 32, 512, 2048
N_CORES = 8
B_SH = B // N_CORES  # 4 batches per core
P = 128              # SBUF partitions
N_CB = C // P        # 4 channel blocks
N_TILES = B_SH * N_CB
LH = L // 2          # 1024 scan columns per tile
LQ = LH // 4         # 256-column quarters for the last tile

_F32 = mybir.dt.float32
_F16 = mybir.dt.float16
_NP_F16 = np.float16

mult = mybir.AluOpType.mult
add = mybir.AluOpType.add

LAST = N_TILES - 1


def build_nc() -> bass.Bass:
    # Bacc (not raw Bass): its compile() runs generate_event_semaphores,
    # which splits multi-sem waits — TRN2 allows at most one wait command
    # per instruction, and Tile freely emits several.
    nc = bacc.Bacc()
    x = nc.dram_tensor("x", [P, N_TILES * L], _F16, kind="ExternalInput")
    # w[p, (cb*3+j)*P + m]: diag weight blocks, j=0: diag(a), 1: diag(a*d),
    # 2: diag(d) for channel block cb (built on host, already in SBUF layout)
    w = nc.dram_tensor("w", [P, N_CB * 3 * P], _F16, kind="ExternalInput")
    d2 = nc.dram_tensor("d2", [P, N_CB], _F32, kind="ExternalInput")
    y = nc.dram_tensor("y", [P, N_TILES * L], _F16, kind="ExternalOutput")

    with TileContext(nc) as tc:
        with (
            tc.tile_pool(name="xp", bufs=5) as xp,
            tc.tile_pool(name="yp", bufs=6) as yp,
            tc.tile_pool(name="cp", bufs=1) as cp,
            tc.tile_pool(name="up", bufs=2, space="PSUM") as up,
            tc.tile_pool(name="wp", bufs=2, space="PSUM") as wp,
        ):
            xv = [None] * N_TILES  # per-tile [P, L] x views (ev | od)
            yv = [None] * N_TILES  # per-tile [P, L] y views
            tiles_u = [None] * N_TILES
            tiles_w = [None] * N_TILES

            def alloc(n):
                yv[n] = yp.tile([P, L], _F16, tag="y", name=f"yt{n}", bufs=8)
                tiles_u[n] = up.tile([P, LH], _F32, tag="u", name="u")
                tiles_w[n] = wp.tile([P, LH], _F32, tag="w", name="wv")

            def load(n, engine, k=1):
                # every tile gets its own buffer: a load trigger that waits
                # for a pool release blocks its whole DMA queue, and the
                # resulting load-lag feeds back into scan-lag
                g = xp.tile(
                    [P, k * L], _F16, tag=f"x{k}", name=f"xt{n}",
                    bufs=(10 if k == 1 else 3),
                )
                engine.dma_start(out=g, in_=x[:, n * L : (n + k) * L])
                for i in range(k):
                    xv[n + i] = g[:, i * L : (i + 1) * L]

            # ---- startup. Each queue serializes trigger+transfer AND a
            # ~2.2us/DMA completion pipeline, and no completion lands before
            # ~12us - so the critical first inputs are spread across all
            # three DMA paths, one deep each:
            #   sync:   d2 (gates scan_0), then x1, x2, ...
            #   swdge:  weights (gate BCA_0)
            #   ring:   tile-0's two interleaved halves
            d2t = cp.tile([P, N_CB], _F32, tag="d2t", name="d2t")
            nc.sync.dma_start(out=d2t, in_=d2[:, :])
            wt = cp.tile([P, N_CB * 3 * P], _F16, tag="wt", name="wt")
            nc.gpsimd.dma_start(out=wt, in_=w[:, :])
            g0 = xp.tile([P, L], _F16, tag="x1", name="xt0", bufs=10)
            xv[0] = g0
            nc.scalar.dma_start(out=g0[:, 0:LH], in_=x[:, 0:LH])
            nc.scalar.dma_start(out=g0[:, LH:L], in_=x[:, LH:L])
            # warm-up ACT op after the triggers: its d2-completion wait must
            # not stall the queued x0 loads; it still runs before copy_0
            warm = cp.tile([P, N_CB], _F32, tag="warm", name="warm")
            nc.scalar.mul(warm, d2t, 1.0)
            # SP queue: x1, x2, singles 3..9, then pairs to compress the tail
            load(1, nc.sync)
            load(2, nc.sync)
            for n in range(3, 10):
                load(n, nc.sync)
            load(10, nc.sync, k=2)
            load(12, nc.sync, k=2)
            load(14, nc.sync, k=2)

            def W(cb, j):
                o = (cb * 3 + j) * P
                return wt[:, o : o + P]

            HB = LH // 2  # 512-column halves

            # tile-15's y block and tile-0's x block are half-interleaved
            # [ev_h0 | od_h0 | ev_h1 | od_h1]; others are [ev | od]
            def zcols(n, lo, hi):
                if n == LAST:
                    k = lo // HB
                    assert (hi - 1) // HB == k
                    return yv[n][:, lo + k * HB : hi + k * HB]
                return yv[n][:, lo:hi]

            def ocols(n, lo, hi):
                if n == LAST:
                    k = lo // HB
                    assert (hi - 1) // HB == k
                    return yv[n][:, lo + (k + 1) * HB : hi + (k + 1) * HB]
                return yv[n][:, LH + lo : LH + hi]

            def xev(n, lo, hi):
                if n == 0:
                    k = lo // HB
                    assert (hi - 1) // HB == k
                    return xv[n][:, lo + k * HB : hi + k * HB]
                return xv[n][:, lo:hi]

            def xod(n, lo, hi):
                if n == 0:
                    k = lo // HB
                    assert (hi - 1) // HB == k
                    return xv[n][:, lo + (k + 1) * HB : hi + (k + 1) * HB]
                return xv[n][:, LH + lo : LH + hi]

            def emit_bca(n, lo, hi):
                """u[lo:hi] = diag(a)@x_ev[lo:hi] + diag(ad)@x_od[lo-1:hi-1]
                (+ diag(d)@x_0 on column 0), emitted as <=512-col matmuls."""
                u = tiles_u[n]
                cb = n // B_SH
                Wa, Wad, Wd = W(cb, 0), W(cb, 1), W(cb, 2)
                for s in range(lo, hi, 512):
                    e = min(s + 512, hi)
                    nc.tensor.matmul(
                        out=u[:, s:e], lhsT=Wa, rhs=xev(n, s, e),
                        start=True, stop=False,
                    )
                for s in range(lo, hi, 512):
                    e = min(s + 512, hi)
                    s2 = max(s, 1)  # u_0 has no x_{-1} term
                    # the shifted odd read od[s2-1:e-1] may straddle an
                    # interleave boundary on tile 0: emit per straddle piece.
                    # stop on the final writer of each PSUM region: regions
                    # containing column 0 are finished by the A-matmul below
                    pieces = [(s2, e)]
                    if n == 0 and (s2 - 1) // HB != (e - 2) // HB:
                        mid = ((s2 - 1) // HB + 1) * HB + 1
                        pieces = [(s2, mid), (mid, e)]
                    for i, (ps, pe) in enumerate(pieces):
                        nc.tensor.matmul(
                            out=u[:, ps:pe], lhsT=Wad,
                            rhs=xod(n, ps - 1, pe - 1),
                            start=False,
                            stop=(s > 0) and (i == len(pieces) - 1),
                        )
                if lo == 0:
                    nc.tensor.matmul(
                        out=u[:, 0:1], lhsT=Wd, rhs=xev(n, 0, 1),
                        start=False, stop=True,
                    )

            def emit_scan(n, lo, hi):
                cb = n // B_SH
                if lo == 0:
                    init = 0.0
                else:
                    init = zcols(n, lo - 1, lo)
                nc.vector.tensor_tensor_scan(
                    out=zcols(n, lo, hi),
                    data0=d2t[:, cb : cb + 1].broadcast_to([P, hi - lo]),
                    data1=tiles_u[n][:, lo:hi],
                    initial=init,
                    op0=mult,
                    op1=add,
                )

            def emit_de(n, lo, hi):
                """wv[lo:hi] = diag(d) @ z[lo:hi] + diag(a) @ x_od[lo:hi]"""
                wv = tiles_w[n]
                cb = n // B_SH
                Wa, Wd = W(cb, 0), W(cb, 2)
                for s in range(lo, hi, 512):
                    e = min(s + 512, hi)
                    nc.tensor.matmul(
                        out=wv[:, s:e], lhsT=Wd, rhs=zcols(n, s, e),
                        start=True, stop=False,
                    )
                for s in range(lo, hi, 512):
                    e = min(s + 512, hi)
                    nc.tensor.matmul(
                        out=wv[:, s:e], lhsT=Wa, rhs=xod(n, s, e),
                        start=False, stop=True,
                    )

            def emit_copy(n, lo, hi):
                nc.scalar.copy(ocols(n, lo, hi), tiles_w[n][:, lo:hi])

            def emit_store(m):
                # spread stores across queues; the LAST stores use sync+ring
                # (one each) so their ~2.2us completion handling overlaps
                if m == N_TILES - 3:
                    dma = nc.sync.dma_start  # sync is done loading by now
                elif m % 2 == 1 or m == N_TILES - 2:
                    dma = nc.scalar.dma_start
                else:
                    dma = nc.gpsimd.dma_start
                dma(out=y[:, m * L : (m + 1) * L], in_=yv[m])

            # tiles 0 and 1: chained half-chunks (the pipeline head is
            # gated by per-DMA completion latency; half-chunks let each
            # scan start one half-load earlier)
            alloc(0)
            for c in range(2):
                lo, hi = c * HB, (c + 1) * HB
                emit_bca(0, lo, hi)
                emit_scan(0, lo, hi)

            for n in range(1, LAST):
                alloc(n)
                if n == 1:
                    for c in range(2):
                        lo, hi = c * HB, (c + 1) * HB
                        emit_bca(n, lo, hi)
                        emit_scan(n, lo, hi)
                    emit_de(n - 1, 0, LH)
                    emit_copy(n - 1, 0, LH)
                    emit_store(n - 1)
                    continue
                emit_bca(n, 0, LH)
                emit_de(n - 1, 0, LH)
                emit_scan(n, 0, LH)
                emit_copy(n - 1, 0, LH)
                emit_store(n - 1)

            # last tile: two chained half-chunks; each half's [ev|od]
            # block stores contiguously on the ACT ring, and each half's
            # odd PSUM region is a whole bank (quarter-granularity DE/copy
            # serialized on PSUM bank accumulation groups)
            alloc(LAST)
            emit_bca(LAST, 0, LH)
            emit_de(LAST - 1, 0, LH)
            emit_scan(LAST, 0, HB)
            emit_scan(LAST, HB, LH)
            emit_copy(LAST - 1, 0, LH)
            emit_store(LAST - 1)
            for k in range(2):
                lo, hi = k * HB, (k + 1) * HB
                emit_de(LAST, lo, hi)
                emit_copy(LAST, lo, hi)
                dma = nc.sync.dma_start if k == 0 else nc.scalar.dma_start
                dma(
                    out=y[:, LAST * L + 2 * lo : LAST * L + 2 * hi],
                    in_=yv[LAST][:, 2 * lo : 2 * hi],
                )

    nc.compile()
    return nc


def _host_consts(alpha: np.ndarray):
    """Diag weight blocks (bf16, SBUF layout) + d^2 (fp32) from alpha."""
    a = alpha[0].astype(np.float64)  # [C]
    d = 1.0 - a
    # fp16 diag entries; d16 = 1 - a16 in bf16 arithmetic so the u_0 column
    # fixup (a16 + d16) lands as close to exactly 1 as fp16 allows
    a16 = a.astype(_NP_F16)
    d16 = (_NP_F16(1.0) - a16).astype(_NP_F16)
    ad16 = (a16 * d16).astype(_NP_F16)
    w = np.zeros((P, N_CB * 3 * P), dtype=_NP_F16)
    idx = np.arange(P)
    for cb in range(N_CB):
        s = slice(cb * P, (cb + 1) * P)
        for j, v in enumerate((a16[s], ad16[s], d16[s])):
            w[idx, (cb * 3 + j) * P + idx] = v
    # d2 pre-arranged to the device SBUF layout [P, N_CB] (column j =
    # channel block j), so the load is one contiguous DMA
    d2 = np.ascontiguousarray(
        (d * d).astype(np.float32).reshape(N_CB, P).T
    )
    return w, d2


def _pack_core(xc: np.ndarray) -> np.ndarray:
    """[B_SH, C, L] -> [P, N_TILES*L] fp16: tile n = (cb*B_SH + b), block
    layout [evens | odds]."""
    x5 = xc.reshape(B_SH, N_CB, P, LH, 2)        # b, cb, p, m, parity
    x5 = x5.transpose(2, 1, 0, 4, 3)             # p, cb, b, parity, m
    out = np.ascontiguousarray(
        x5.astype(_NP_F16)
    ).reshape(P, N_TILES * L)
    # tile 0's block becomes half-interleaved [ev_h0|od_h0|ev_h1|od_h1]
    t0 = out[:, 0:L].reshape(P, 2, 2, LH // 2)   # p, parity, half, m
    out[:, 0:L] = np.ascontiguousarray(
        t0.transpose(0, 2, 1, 3)
    ).reshape(P, L)
    return out


def _unpack_core(yc: np.ndarray) -> np.ndarray:
    """Inverse of _pack_core (+ the last tile's quarter-interleaved block)."""
    yc = yc.reshape(P, N_TILES, L).astype(np.float32)
    std = np.empty((P, N_TILES, 2, LH), dtype=np.float32)
    std[:, :, 0, :] = yc[:, :, 0:LH]
    std[:, :, 1, :] = yc[:, :, LH:L]
    lasth = yc[:, LAST].reshape(P, 2, 2, LH // 2)  # p, half, parity, m
    std[:, LAST, 0] = lasth[:, :, 0].reshape(P, LH)
    std[:, LAST, 1] = lasth[:, :, 1].reshape(P, LH)
    # std: p, (cb b), parity, m  ->  b, cb*P+p, 2m+parity
    y5 = std.reshape(P, N_CB, B_SH, 2, LH).transpose(2, 1, 0, 4, 3)
    return np.ascontiguousarray(y5).reshape(B_SH, C, L)


_cached_nc = None


def _get_nc() -> bass.Bass:
    global _cached_nc
    if _cached_nc is None:
        _cached_nc = build_nc()
    return _cached_nc


def run(x: np.ndarray, alpha: np.ndarray, nc=None, **spmd_kwargs):
    """Full host path: prep inputs, run on 8 cores, reassemble output.
    Returns (y, BassKernelResults)."""
    assert x.shape == (B, C, L) and alpha.shape == (1, C)
    x = np.asarray(x, dtype=np.float32)
    alpha = np.ascontiguousarray(alpha, dtype=np.float32)
    w, d2 = _host_consts(alpha)
    if nc is None:
        nc = _get_nc()
    in_maps = [
        {"x": _pack_core(x[c * B_SH : (c + 1) * B_SH]), "w": w, "d2": d2}
        for c in range(N_CORES)
    ]
    res = run_bass_kernel_spmd(nc, in_maps, list(range(N_CORES)), **spmd_kwargs)
    y = np.concatenate([_unpack_core(r["y"]) for r in res.results], axis=0)
    return y, res


def kernel(x: np.ndarray, alpha: np.ndarray) -> np.ndarray:
    return run(x, alpha)[0]
